# revision 57
# baseline (speedup 1.0000x reference)
"""DenseGAT layer (LN -> masked 12-head attention -> LN -> GELU FFN, residuals)
on 8 Trainium2 NeuronCores, data-parallel over the batch (4 graphs/core).

Math notes (validated against the reference in fp64/numpy):
- LN affine (g,b) is folded into the following projection weights on the host,
  along with the 1/sqrt(hd) attention scale (into wq) and 0-biases.
- Scores are computed directly transposed, scoresT[m,n] = kT_h^T-slice @ qT_h,
  so softmax's reduction lands on the matmul contraction axis: masked exp
  weights eT are multiplied by the host-transposed 0/1 mask, and the
  denominator comes for free as a 65th "ones" column of V. No max-subtraction
  is needed: |scores| < ~3 for this model family (exp is safe in fp32).
- All matmuls run in bf16 with fp32 PSUM accumulation; LN statistics and
  softmax normalization stay in fp32; x/out/outres are fp16 (residual
  quantization ~5e-4, well inside the 2e-2 gate).
"""
import numpy as np
import ml_dtypes

import concourse.bass as bass
import concourse.mybir as mybir
import concourse.tile as tile
from concourse.bass_utils import run_bass_kernel_spmd

bf16 = ml_dtypes.bfloat16
FP32 = mybir.dt.float32
FP16 = mybir.dt.float16
BF16 = mybir.dt.bfloat16
F = mybir.ActivationFunctionType
OP = mybir.AluOpType

B, N, D, H = 32, 512, 768, 12
HD = D // H            # 64
NCORES = 8
GPC = B // NCORES      # graphs per core
NT = N // 128          # 4 n-tiles (also m-chunks)
DC = D // 128          # 6 d-chunks
FC = 4 * D // 128      # 24 ffn chunks
EPS = 1e-5


def _split_waits(nc):
    """This walrus accepts one sync-wait per instruction (two for
    EventSemaphore); hoist excess waits onto same-engine nops inserted before
    the instruction (engines dispatch in order, so semantics are preserved)."""
    counter = 0
    for f in nc.m.functions:
        for blk in f.blocks:
            insts = blk.instructions
            i = 0
            while i < len(insts):
                inst = insts[i]
                si = getattr(inst, "sync_info", None)
                waits = list(si.on_wait) if si is not None and si.on_wait else []
                cap = 2 if isinstance(inst, mybir.InstEventSemaphore) else 1
                if len(waits) > cap:
                    si.on_wait = waits[-cap:]
                    for c in waits[:-cap]:
                        counter += 1
                        nop = mybir.InstNoOp(name=f"wsplit-{counter}", ins=[], outs=[])
                        nop.engine = inst.engine
                        nop.sync_info = type(si)(on_wait=[c], on_update=[])
                        insts.insert(i, nop)
                        i += 1
                i += 1


# all weights, biases and the per-core transposed mask (bf16) live in ONE 1-D
# blob: every extra PJRT argument costs measurable per-exec dispatch time
# through the tunnel, so the kernel takes just two inputs (x4 fp16 + blob).
_BF16_SPEC = [
    ("wq", D * D), ("wk", D * D), ("wv", D * D), ("wo", D * D),
    ("w1", FC * 128 * D), ("w2", 4 * D * D),
    ("bv", D), ("bo", D), ("b2f", D),
    ("bq", D), ("bk", D), ("b1f", 4 * D),
    ("maskT4", GPC * N * N),
]
_BF16_OFF = {}
_off = 0
for _nm, _sz in _BF16_SPEC:
    _BF16_OFF[_nm] = _off
    _off += _sz
_BF16_TOT = _off


def _build_module():
    nc = bass.Bass()
    dram = {
        "x4": nc.dram_tensor("x4", [GPC, N, D], FP16, kind="ExternalInput"),
        "wb": nc.dram_tensor("wb", [_BF16_TOT], BF16, kind="ExternalInput"),
    }
    out4 = nc.dram_tensor("out4", [GPC, N, D], FP16, kind="ExternalOutput")

    with tile.TileContext(nc) as tc:
        _emit(nc, tc, dram, out4)
    _split_waits(nc)
    return nc


def _wb(dram, name, sz, extra_off=0):
    return dram["wb"][_BF16_OFF[name] + extra_off:
                      _BF16_OFF[name] + extra_off + sz]


def _emit(nc, tc, dram, out4):
    import contextlib
    ctx = contextlib.ExitStack()
    with ctx:
        res = ctx.enter_context(tc.tile_pool(name="res", bufs=1))       # resident
        w1s = ctx.enter_context(tc.tile_pool(name="w1s", bufs=7))       # streamed w1 chunks
        xp = ctx.enter_context(tc.tile_pool(name="xp", bufs=2))
        mkp = ctx.enter_context(tc.tile_pool(name="mkp", bufs=1))
        znat = ctx.enter_context(tc.tile_pool(name="znat", bufs=2))     # z / ao / z2
        ztp = ctx.enter_context(tc.tile_pool(name="ztp", bufs=2))       # zT / aoT / z2T
        qkp = ctx.enter_context(tc.tile_pool(name="qkp", bufs=1))       # qT, kT
        vp = ctx.enter_context(tc.tile_pool(name="vp", bufs=1))
        etp = ctx.enter_context(tc.tile_pool(name="etp", bufs=7))
        gp = ctx.enter_context(tc.tile_pool(name="gp", bufs=1))         # gT resident per graph
        orp = ctx.enter_context(tc.tile_pool(name="orp", bufs=1))       # outres
        fin = ctx.enter_context(tc.tile_pool(name="fin", bufs=2))
        sm = ctx.enter_context(tc.tile_pool(name="sm", bufs=4))         # small stats tiles
        mm = ctx.enter_context(tc.tile_pool(name="mm", bufs=5, space="PSUM"))
        av = ctx.enter_context(tc.tile_pool(name="av", bufs=3, space="PSUM"))

        # ---- input prefetch (graph 0 first so weight loads don't delay it) --
        x_tiles, mk_tiles = {}, {}
        zTs, qTs, kTs, vsbs = {}, {}, {}, {}

        def xsrc(b):
            return dram["x4"][b].rearrange("(t p) d -> p t d", p=128)

        def load_x(b):
            x_tiles[b] = xp.tile([128, NT, D], FP16, name=f"x{b}", tag="x")
            nc.sync.dma_start(out=x_tiles[b], in_=xsrc(b))

        def load_mask(b):
            mk_tiles[b] = mkp.tile([128, NT, N], BF16, name=f"mk{b}", tag="mk")
            nc.sync.dma_start(
                out=mk_tiles[b],
                in_=_wb(dram, "maskT4", N * N, extra_off=b * N * N)
                .rearrange("(c p n) -> p c n", p=128, n=N))

        # ---- one-time loads -------------------------------------------------
        def load_w(name, chunks, cols):
            t = res.tile([128, chunks, cols], BF16, tag=name)
            nc.sync.dma_start(
                out=t, in_=_wb(dram, name, chunks * 128 * cols)
                .rearrange("(c p d) -> p c d", p=128, d=cols))
            return t

        # startup: the serial DMA pipe is the bottleneck, so order it by need:
        # all 4 x tiles, then wq's first column-half (enough for qk j=0..2),
        # then the rest. The zT0 transposes ride the ACT HWDGE queue so their
        # data-waits don't block later SP configs; per-tile LN runs as each x
        # tile lands.
        eps_t = res.tile([128, 1], FP32, tag="eps")
        nc.vector.memset(eps_t, EPS)
        x_tiles[0] = xp.tile([128, NT, D], FP16, name="x0", tag="x")
        x0src = xsrc(0)
        z0_sb = znat.tile([128, NT, D], BF16, tag="znat", name="z0")
        zT0 = ztp.tile([128, DC, N], BF16, tag="zt", name="zT0")
        for t_ in range(NT):
            nc.sync.dma_start(out=x_tiles[0][:, t_, :], in_=x0src[:, t_, :])
        # wq first column-half first (cols j<3 of every dc chunk, enough for
        # qk j=0..2), then the rest after the LN chain is queued
        wq_sb = res.tile([128, DC, D], BF16, tag="wq")
        wq_src = _wb(dram, "wq", DC * 128 * D).rearrange(
            "(c p d) -> p c d", p=128, d=D)
        nc.sync.dma_start(out=wq_sb[:, :, 0:384], in_=wq_src[:, :, 0:384])
        for t_ in range(NT):
            _layernorm_tile(nc, sm, x_tiles[0], z0_sb, eps_t, t_)
            nc.scalar.dma_start_transpose(out=zT0[:, :, t_ * 128:(t_ + 1) * 128],
                                          in_=z0_sb[:, t_, :])
        zTs[0] = zT0
        nc.sync.dma_start(out=wq_sb[:, :, 384:768], in_=wq_src[:, :, 384:768])
        wk_sb = load_w("wk", DC, D)
        wv_sb = load_w("wv", DC, D)
        load_mask(0)

        bq_sb = res.tile([128, DC], BF16, tag="bq")
        nc.sync.dma_start(out=bq_sb,
                          in_=_wb(dram, "bq", D).rearrange("(c p) -> p c", p=128))
        bk_sb = res.tile([128, DC], BF16, tag="bk")
        nc.sync.dma_start(out=bk_sb,
                          in_=_wb(dram, "bk", D).rearrange("(c p) -> p c", p=128))
        b1f_sb = res.tile([128, FC], BF16, tag="b1f")
        nc.sync.dma_start(out=b1f_sb,
                          in_=_wb(dram, "b1f", 4 * D).rearrange("(c p) -> p c", p=128))

        def bcast(name):
            t = res.tile([128, D], BF16, tag=name + "b")
            src = _wb(dram, name, D)
            nc.sync.dma_start(out=t, in_=bass.AP(
                tensor=src.tensor, offset=src.offset, ap=[[0, 128]] + list(src.ap)))
            return t

        bv_b = bcast("bv")
        bo_b = bcast("bo")
        b2_b = bcast("b2f")

        # ---- graph-level software pipeline ---------------------------------
        # LN1/zT and QKV for graph b+1 are emitted inside graph b's body so
        # each engine's static instruction order interleaves next-graph work
        # into this graph's stall windows.

        def ln_z_zT(b):
            z_sb = znat.tile([128, NT, D], BF16, tag="znat", name=f"z{b}")
            zT = ztp.tile([128, DC, N], BF16, tag="zt", name=f"zT{b}")
            _layernorm(nc, sm, x_tiles[b], z_sb, eps_t)
            for t in range(NT):
                nc.sync.dma_start_transpose(
                    out=zT[:, :, t * 128:(t + 1) * 128], in_=z_sb[:, t, :])
            zTs[b] = zT

        def qk_proj(b, split_n=False):
            zT = zTs[b]
            qT = qkp.tile([128, DC, N], BF16, tag="q", name=f"qT{b}")
            kT = qkp.tile([128, DC, N], BF16, tag="k", name=f"kT{b}")
            nsplits = ((0, 256), (256, 512)) if split_n else ((0, 512),)
            jgroups = (range(DC),)
            for jg in jgroups:
                for wsb, bsb, outT in ((wq_sb, bq_sb, qT), (wk_sb, bk_sb, kT)):
                    for j in jg:
                        ps = mm.tile([128, N], FP32, tag="mm")
                        for n0, n1 in nsplits:
                            for dc in range(DC):
                                nc.tensor.matmul(ps[:, n0:n1],
                                                 wsb[:, dc, j * 128:(j + 1) * 128],
                                                 zT[:, dc, n0:n1],
                                                 start=(dc == 0),
                                                 stop=(dc == DC - 1))
                        nc.scalar.activation(out=outT[:, j, :], in_=ps[:, :],
                                             func=F.Identity, bias=bsb[:, j:j + 1])
            qTs[b], kTs[b] = qT, kT

        def v_proj(b):
            zT = zTs[b]
            v_sb = vp.tile([128, NT, H, HD + 1], BF16, name=f"v{b}", tag="v")
            for mc in range(NT):
                for half in range(2):
                    ps = mm.tile([128, 384], FP32, tag="mm")
                    for dc in range(DC):
                        nc.tensor.matmul(ps[:, :],
                                         zT[:, dc, mc * 128:(mc + 1) * 128],
                                         wv_sb[:, dc, half * 384:(half + 1) * 384],
                                         start=(dc == 0), stop=(dc == DC - 1))
                    nc.vector.tensor_tensor(
                        out=v_sb[:, mc, half * 6:(half + 1) * 6, 0:HD],
                        in0=ps.rearrange("p (h d) -> p h d", d=HD),
                        in1=bv_b[:, half * 384:(half + 1) * 384].rearrange(
                            "p (h d) -> p h d", d=HD),
                        op=OP.add)
            nc.vector.memset(v_sb[:, :, :, HD:HD + 1], 1.0)
            vsbs[b] = v_sb

        eTs_pre = {}

        def scores_pair(p, qT, kT, mk_sb, gtag):
            eTp = [etp.tile([128, NT, N], BF16, tag="et", name=f"eT{gtag}_{p}_{s_}")
                   for s_ in range(2)]
            for mc in range(NT):
                for sub in range(2):
                    off = sub * 64
                    ps = mm.tile([128, N], FP32, tag="mm")
                    nc.tensor.matmul(
                        ps[:, :],
                        kT[off:off + 64, p, mc * 128:(mc + 1) * 128],
                        qT[off:off + 64, p, :],
                        start=True, stop=True)
                    nc.scalar.activation(out=eTp[sub][:, mc, :], in_=ps[:, :],
                                         func=F.Exp)
                    nc.vector.tensor_tensor(out=eTp[sub][:, mc, :],
                                            in0=eTp[sub][:, mc, :],
                                            in1=mk_sb[:, mc, :], op=OP.mult)
            return eTp

        qk_proj(0, split_n=True)
        v_proj(0)
        # x1 isn't needed until LN1(1) during graph 0's attention (~30us);
        # keep its transfer out of the startup pipe
        with tc.tile_wait_until(0.012):
            load_x(1)
        # wo/w2 aren't needed until O-proj/FFN2 of graph 0 (~90us in); keep
        # their transfers out of the startup DMA-pipe rush
        with tc.tile_wait_until(0.018):
            wo_sb = load_w("wo", DC, D)
            w2_sb = load_w("w2", FC, D)

        # w1 chunk streaming: deep prefetch on the ACT DMA queue (the SP
        # queue is congested with transposes/stores, which made chunks late)
        W1_LOOKAHEAD = 6

        def load_w1c(fc, tiles, hold=False):
            # SWDGE via gpsimd: stays off the HWDGE generators that serialize
            # the SP/ACT queues' transposes, stores and input loads. `hold`
            # keeps graph 0's prefetches out of the startup DMA-pipe rush
            # (without a dep they'd be scheduled at t=0).
            t = w1s.tile([128, DC, 128], BF16, tag="w1s")
            with tc.tile_wait_until(0.025, enable=hold):
                nc.gpsimd.dma_start(
                    out=t, in_=_wb(dram, "w1", 128 * D, extra_off=fc * 128 * D)
                    .rearrange("(p c f) -> p c f", p=128, f=128))
            tiles.append(t)

        for b in range(GPC):
            x_sb = x_tiles[b]
            mk_sb = mk_tiles.pop(b)
            qT, kT, v_sb = qTs.pop(b), kTs.pop(b), vsbs.pop(b)

            # ---- attention: 6 head-pairs, software-pipelined ----
            ao = znat.tile([128, NT, D], BF16, tag="znat", name=f"ao{b}")
            pav = {}

            def scores_block(p):
                return scores_pair(p, qT, kT, mk_sb, b)

            def av_block(p, eTp):
                half = p // 3
                if half not in pav:
                    pav[half] = [av.tile([128, 6 * (HD + 1)], FP32, tag="av",
                                         name=f"pav{half}_{nc4i}")
                                 for nc4i in range(NT)]
                for sub in range(2):
                    h = 2 * p + sub
                    lane = h - half * 6
                    for nc4 in range(NT):
                        for mc in range(NT):
                            nc.tensor.matmul(
                                pav[half][nc4][:, lane * 65:lane * 65 + 65],
                                eTp[sub][:, mc, nc4 * 128:(nc4 + 1) * 128],
                                v_sb[:, mc, h, :],
                                start=(mc == 0), stop=(mc == NT - 1))

            def evac_block(half):
                for nc4 in range(NT):
                    rec = sm.tile([128, 6], FP32, tag="rec")
                    nc.vector.reciprocal(out=rec, in_=pav[half][nc4][:, HD::HD + 1])
                    rb = bass.AP(tensor=rec.tensor, offset=rec.offset,
                                 ap=[rec.ap[0], [rec.ap[1][0], 6], [0, HD]])
                    nc.vector.tensor_tensor(
                        out=ao[:, nc4, half * 384:(half + 1) * 384].rearrange(
                            "p (h d) -> p h d", d=HD),
                        in0=pav[half][nc4].rearrange(
                            "p (h s) -> p h s", s=HD + 1)[:, :, 0:HD],
                        in1=rb, op=OP.mult)

            pre = eTs_pre.pop(b, None)
            if pre:
                queue = list(pre)
                nxt = len(pre)
            else:
                queue = [scores_block(0)]
                nxt = 1
            for p in range(6):
                if nxt < 6:
                    queue.append(scores_block(nxt))
                    nxt += 1
                eT_cur = queue.pop(0)
                av_block(p, eT_cur)
                if p == 0 and b + 1 < GPC:
                    # next graph's LN1+transpose: its DVE/ACT/DMA work fills
                    # this graph's attention slack, and zT(b+1) is ready before
                    # the post-attention QK(b+1) matmuls need it.
                    ln_z_zT(b + 1)
                if p == 2:
                    evac_block(0)
                elif p == 5:
                    evac_block(1)
                    if b + 1 < GPC:
                        load_mask(b + 1)

            # x += bo (residual bias; gpsimd is free again here)
            for t in range(NT):
                nc.gpsimd.tensor_tensor(out=x_sb[:, t, :], in0=x_sb[:, t, :],
                                        in1=bo_b, op=OP.add)

            # aoT
            aoT = ztp.tile([128, DC, N], BF16, tag="zt", name=f"aoT{b}")
            for t in range(NT):
                nc.sync.dma_start_transpose(out=aoT[:, :, t * 128:(t + 1) * 128],
                                            in_=ao[:, t, :])

            # next graph's q/k projections fill the aoT-transpose latency
            if b + 1 < GPC:
                qk_proj(b + 1)

            # prefetch first w1 chunks (Pool SWDGE queue)
            w1_tiles = []
            for fc in range(W1_LOOKAHEAD):
                load_w1c(fc, w1_tiles, hold=(b == 0))

            # O projection + residual -> outres (fp16), LN2 interleaved per
            # tile so z2T(t) transposes overlap O-proj of tiles t+1..3 (the
            # DVE/ACT LN2 work for tile t queues before tile t+1's evac).
            outres = orp.tile([128, NT, D], FP16, name=f"or{b}", tag="or")
            sums = sm.tile([128, NT, 2], FP32, tag="s1")
            sumsq = sm.tile([128, NT, 2], FP32, tag="s2")
            z2 = znat.tile([128, NT, D], BF16, tag="znat", name=f"z2{b}")
            z2T = ztp.tile([128, DC, N], BF16, tag="zt", name=f"z2T{b}")
            for nc4 in range(NT):
                for half in range(2):
                    ps = mm.tile([128, 384], FP32, tag="mm")
                    for dc in range(DC):
                        nc.tensor.matmul(ps[:, :],
                                         aoT[:, dc, nc4 * 128:(nc4 + 1) * 128],
                                         wo_sb[:, dc, half * 384:(half + 1) * 384],
                                         start=(dc == 0), stop=(dc == DC - 1))
                    # evacuation + residual, with a free row-sum for LN2 stats
                    nc.vector.scalar_tensor_tensor(
                        out=outres[:, nc4, half * 384:(half + 1) * 384],
                        in0=ps[:, :], scalar=0.0,
                        in1=x_sb[:, nc4, half * 384:(half + 1) * 384],
                        op0=OP.add, op1=OP.add,
                        accum_out=sums[:, nc4, half:half + 1])
                    # row sum of squares on the scalar engine (idle here)
                    sq = fin.tile([128, 384], FP32, tag="fin")
                    nc.scalar.activation(
                        out=sq, in_=outres[:, nc4, half * 384:(half + 1) * 384],
                        func=F.Square,
                        accum_out=sumsq[:, nc4, half:half + 1])
                # LN2(tile nc4): mu = s/D, var = sq/D - mu^2
                t = nc4
                mu = sm.tile([128, 1], FP32, tag="mv")
                nc.vector.tensor_reduce(out=mu, in_=sums[:, t, :],
                                        axis=mybir.AxisListType.X, op=OP.add)
                nc.vector.tensor_scalar(out=mu, in0=mu, scalar1=1.0 / D,
                                        scalar2=None, op0=OP.mult)
                ex2 = sm.tile([128, 1], FP32, tag="lnv")
                nc.vector.tensor_reduce(out=ex2, in_=sumsq[:, t, :],
                                        axis=mybir.AxisListType.X, op=OP.add)
                nc.vector.tensor_scalar(out=ex2, in0=ex2, scalar1=1.0 / D,
                                        scalar2=None, op0=OP.mult)
                negvar = sm.tile([128, 1], FP32, tag="bn")
                nc.vector.scalar_tensor_tensor(
                    out=negvar, in0=mu, scalar=mu, in1=ex2,
                    op0=OP.mult, op1=OP.subtract)   # mu^2 - E[x^2] = -var
                lnv = sm.tile([128, 1], FP32, tag="lnv2")
                # Ln(-1 * negvar + eps) = Ln(var + eps)
                nc.scalar.activation(out=lnv, in_=negvar, func=F.Ln, bias=eps_t,
                                     scale=-1.0)
                rstd = sm.tile([128, 1], FP32, tag="rstd")
                nc.scalar.activation(out=rstd, in_=lnv, func=F.Exp, scale=-0.5)
                nc.vector.tensor_scalar(out=z2[:, t, :], in0=outres[:, t, :],
                                        scalar1=mu, scalar2=rstd,
                                        op0=OP.subtract, op1=OP.mult)
                # ACT queue: fires right after this tile's rstd instead of
                # queuing behind unrelated SP configs
                nc.scalar.dma_start_transpose(
                    out=z2T[:, :, t * 128:(t + 1) * 128], in_=z2[:, t, :])

            # next graph's v projection fills the LN2/z2T stall window on the PE
            if b + 1 < GPC:
                v_proj(b + 1)

            # outres += b2 (final-residual bias), off critical path
            for t in range(NT):
                nc.gpsimd.tensor_tensor(out=outres[:, t, :], in0=outres[:, t, :],
                                        in1=b2_b, op=OP.add)

            # FFN1 + gelu -> gT. First chunks run as two n-halves so they only
            # need z2T transposes t=0,1 (t=2,3 still in flight after O-proj).
            gT = gp.tile([128, FC, N], BF16, name=f"gT{b}", tag="gT")
            for fc in range(FC):
                if fc + W1_LOOKAHEAD < FC:
                    load_w1c(fc + W1_LOOKAHEAD, w1_tiles,
                             hold=(b == 0 and fc < 2))
                w1c = w1_tiles[fc]
                ps = mm.tile([128, N], FP32, tag="mm")
                nsp = ((0, 256), (256, 512)) if fc < 4 else ((0, 512),)
                for n0, n1 in nsp:
                    for dc in range(DC):
                        nc.tensor.matmul(ps[:, n0:n1], w1c[:, dc, :],
                                         z2T[:, dc, n0:n1],
                                         start=(dc == 0), stop=(dc == DC - 1))
                nc.scalar.activation(out=gT[:, fc, :], in_=ps[:, :], func=F.Gelu,
                                     bias=b1f_sb[:, fc:fc + 1])
                if b + 1 < GPC and fc in (15, 19):
                    # pre-compute next graph's first score pairs in the FFN
                    # window (ACT has slack beside gelu); the next attention
                    # window then starts PE-bound instead of exp-bound
                    eTs_pre.setdefault(b + 1, []).append(
                        scores_pair(0 if fc == 15 else 1, qTs[b + 1],
                                    kTs[b + 1], mk_tiles[b + 1], b + 1))

            if b + 2 < GPC:
                load_x(b + 2)

            # FFN2 + final residual -> out. ACT is idle here, so pre-compute
            # two more of the next graph's score pairs (exp on ACT) — the
            # next attention window then starts with 4/6 pairs done.
            for nc4 in range(NT):
                for half in range(2):
                    ps = mm.tile([128, 384], FP32, tag="mm")
                    for fc in range(FC):
                        nc.tensor.matmul(ps[:, :],
                                         gT[:, fc, nc4 * 128:(nc4 + 1) * 128],
                                         w2_sb[:, fc, half * 384:(half + 1) * 384],
                                         start=(fc == 0), stop=(fc == FC - 1))
                    ft = fin.tile([128, 384], FP16, tag="fin")
                    nc.vector.tensor_tensor(
                        out=ft, in0=ps[:, :],
                        in1=outres[:, nc4, half * 384:(half + 1) * 384], op=OP.add)
                    nc.sync.dma_start(
                        out=out4[b, nc4 * 128:(nc4 + 1) * 128,
                                 half * 384:(half + 1) * 384],
                        in_=ft)
                if b + 1 < GPC and nc4 in (1, 3):
                    eTs_pre[b + 1].append(
                        scores_pair(2 if nc4 == 1 else 3, qTs[b + 1],
                                    kTs[b + 1], mk_tiles[b + 1], b + 1))
            del x_tiles[b]


def _layernorm_tile(nc, sm, src, dst, eps_t, t):
    """Single-tile LN chain (stats -> rstd -> z) for pipeline fill: no
    cross-tile batching, so tile t's output is ready as soon as tile t's
    input is."""
    stats = sm.tile([128, 3, 6], FP32, tag="bn")
    xg = src[:, t, :].rearrange("p (s d) -> p s d", s=3)
    for s in range(3):
        nc.vector.bn_stats(out=stats[:, s, :], in_=xg[:, s, :])
    mv = sm.tile([128, 2], FP32, tag="mv")
    nc.vector.bn_aggr(out=mv, in_=stats)
    lnv = sm.tile([128, 1], FP32, tag="lnv")
    nc.scalar.activation(out=lnv, in_=mv[:, 1:2], func=F.Ln, bias=eps_t)
    rstd = sm.tile([128, 1], FP32, tag="rstd")
    nc.scalar.activation(out=rstd, in_=lnv, func=F.Exp, scale=-0.5)
    nc.vector.tensor_scalar(out=dst[:, t, :], in0=src[:, t, :],
                            scalar1=mv[:, 0:1], scalar2=rstd,
                            op0=OP.subtract, op1=OP.mult)


def _layernorm(nc, sm, src, dst, eps_t):
    """src [128, NT, 768] fp32 -> dst [128, NT, 768] bf16, per-row LN without
    affine (folded into downstream weights). The Ln/Exp rstd ops are batched
    across all NT tiles (2 ACT ops instead of 2*NT) since this runs inside the
    ACT-bound attention window."""
    mv4 = sm.tile([128, NT, 2], FP32, tag="mv")
    for t in range(NT):
        stats = sm.tile([128, 3, 6], FP32, tag="bn")
        xg = src[:, t, :].rearrange("p (s d) -> p s d", s=3)
        for s in range(3):
            nc.vector.bn_stats(out=stats[:, s, :], in_=xg[:, s, :])
        nc.vector.bn_aggr(out=mv4[:, t, :], in_=stats)
    lnv4 = sm.tile([128, NT], FP32, tag="lnv")
    nc.scalar.activation(out=lnv4, in_=mv4[:, :, 1], func=F.Ln, bias=eps_t)
    rstd4 = sm.tile([128, NT], FP32, tag="rstd")
    nc.scalar.activation(out=rstd4, in_=lnv4, func=F.Exp, scale=-0.5)
    for t in range(NT):
        nc.vector.tensor_scalar(out=dst[:, t, :], in0=src[:, t, :],
                                scalar1=mv4[:, t, 0:1], scalar2=rstd4[:, t:t + 1],
                                op0=OP.subtract, op1=OP.mult)


_CACHE = {}


def _get_module():
    if "nc" not in _CACHE:
        _CACHE["nc"] = _build_module()
    return _CACHE["nc"]


def _prep_inputs(inputs):
    x = np.ascontiguousarray(np.asarray(inputs["x"], dtype=np.float32)
                             .astype(np.float16))
    adj = np.asarray(inputs["adj"])
    g1 = np.asarray(inputs["g1"], dtype=np.float32)
    b1 = np.asarray(inputs["b1"], dtype=np.float32)
    g2 = np.asarray(inputs["g2"], dtype=np.float32)
    b2 = np.asarray(inputs["b2"], dtype=np.float32)
    scale = HD ** -0.5

    wq = np.asarray(inputs["wq"], np.float32)
    wk = np.asarray(inputs["wk"], np.float32)
    wv = np.asarray(inputs["wv"], np.float32)
    w1 = np.asarray(inputs["w_ffn1"], np.float32)

    consts = {
        "wq": ((g1[:, None] * wq) * scale).astype(bf16),
        "wk": (g1[:, None] * wk).astype(bf16),
        "wv": (g1[:, None] * wv).astype(bf16),
        "wo": np.asarray(inputs["wo"], np.float32).astype(bf16),
        "w1": (g2[:, None] * w1).astype(bf16).reshape(6, 128, 24, 128)
              .transpose(2, 1, 0, 3).reshape(24, 128, 768),
        "w2": np.asarray(inputs["w_ffn2"], np.float32).astype(bf16),
        "bq": ((b1 @ wq + np.asarray(inputs["bq"], np.float32)) * scale),
        "bk": (b1 @ wk + np.asarray(inputs["bk"], np.float32)),
        "bv": (b1 @ wv + np.asarray(inputs["bv"], np.float32)).astype(bf16),
        "bo": np.asarray(inputs["bo"], np.float32).astype(bf16),
        "b1f": (b2 @ w1 + np.asarray(inputs["b_ffn1"], np.float32)),
        "b2f": np.asarray(inputs["b_ffn2"], np.float32).astype(bf16),
    }
    maskT = np.ascontiguousarray(adj.transpose(0, 2, 1)).astype(bf16)
    for nm in ("bq", "bk", "b1f"):
        consts[nm] = consts[nm].astype(bf16)

    # assemble the single blob (see _BF16_SPEC)
    wb_const = np.concatenate(
        [np.ascontiguousarray(consts[nm]).ravel()
         for nm, _ in _BF16_SPEC if nm not in ("maskT4", "x4")])

    in_maps = []
    for c in range(NCORES):
        wb = np.concatenate(
            [wb_const, maskT[c * GPC:(c + 1) * GPC].ravel()])
        assert wb.size == _BF16_TOT
        in_maps.append({"x4": x[c * GPC:(c + 1) * GPC], "wb": wb})
    return in_maps


def _run(inputs, **kwargs):
    nc = _get_module()
    in_maps = _prep_inputs(inputs)
    res = run_bass_kernel_spmd(nc, in_maps, core_ids=list(range(NCORES)), **kwargs)
    out = np.concatenate([res.results[c]["out4"] for c in range(NCORES)],
                         axis=0).astype(np.float32)
    return out, res


def kernel(**inputs) -> np.ndarray:
    out, _ = _run(inputs)
    return out


def run_traced(inputs):
    """For test.py: returns (output, BassKernelResults with profile info)."""
    return _run(inputs, trace=True)



# revision 59
# speedup vs baseline: 1.0251x; 1.0251x over previous
"""DenseGAT layer (LN -> masked 12-head attention -> LN -> GELU FFN, residuals)
on 8 Trainium2 NeuronCores, data-parallel over the batch (4 graphs/core).

Math notes (validated against the reference in fp64/numpy):
- LN affine (g,b) is folded into the following projection weights on the host,
  along with the 1/sqrt(hd) attention scale (into wq) and 0-biases.
- Scores are computed directly transposed, scoresT[m,n] = kT_h^T-slice @ qT_h,
  so softmax's reduction lands on the matmul contraction axis: masked exp
  weights eT are multiplied by the host-transposed 0/1 mask, and the
  denominator comes for free as a 65th "ones" column of V. No max-subtraction
  is needed: |scores| < ~3 for this model family (exp is safe in fp32).
- All matmuls run in bf16 with fp32 PSUM accumulation; LN statistics and
  softmax normalization stay in fp32; x/out/outres are fp16 (residual
  quantization ~5e-4, well inside the 2e-2 gate).
"""
import numpy as np
import ml_dtypes

import concourse.bass as bass
import concourse.mybir as mybir
import concourse.tile as tile
from concourse.bass_utils import run_bass_kernel_spmd

bf16 = ml_dtypes.bfloat16
FP32 = mybir.dt.float32
FP16 = mybir.dt.float16
BF16 = mybir.dt.bfloat16
F = mybir.ActivationFunctionType
OP = mybir.AluOpType

B, N, D, H = 32, 512, 768, 12
HD = D // H            # 64
NCORES = 8
GPC = B // NCORES      # graphs per core
NT = N // 128          # 4 n-tiles (also m-chunks)
DC = D // 128          # 6 d-chunks
FC = 4 * D // 128      # 24 ffn chunks
EPS = 1e-5


def _split_waits(nc):
    """This walrus accepts one sync-wait per instruction (two for
    EventSemaphore); hoist excess waits onto same-engine nops inserted before
    the instruction (engines dispatch in order, so semantics are preserved)."""
    counter = 0
    for f in nc.m.functions:
        for blk in f.blocks:
            insts = blk.instructions
            i = 0
            while i < len(insts):
                inst = insts[i]
                si = getattr(inst, "sync_info", None)
                waits = list(si.on_wait) if si is not None and si.on_wait else []
                cap = 2 if isinstance(inst, mybir.InstEventSemaphore) else 1
                if len(waits) > cap:
                    si.on_wait = waits[-cap:]
                    for c in waits[:-cap]:
                        counter += 1
                        nop = mybir.InstNoOp(name=f"wsplit-{counter}", ins=[], outs=[])
                        nop.engine = inst.engine
                        nop.sync_info = type(si)(on_wait=[c], on_update=[])
                        insts.insert(i, nop)
                        i += 1
                i += 1


# all weights, biases and the per-core transposed mask (bf16) live in ONE 1-D
# blob: every extra PJRT argument costs measurable per-exec dispatch time
# through the tunnel, so the kernel takes just two inputs (x4 fp16 + blob).
_BF16_SPEC = [
    ("wq", D * D), ("wk", D * D), ("wv", D * D), ("wo", D * D),
    ("w1", FC * 128 * D), ("w2", 4 * D * D),
    ("bv", D), ("bo", D), ("b2f", D),
    ("bq", D), ("bk", D), ("b1f", 4 * D),
    ("maskT4", GPC * N * N),
]
_BF16_OFF = {}
_off = 0
for _nm, _sz in _BF16_SPEC:
    _BF16_OFF[_nm] = _off
    _off += _sz
_BF16_TOT = _off


def _build_module():
    nc = bass.Bass()
    dram = {
        "x4": nc.dram_tensor("x4", [GPC, N, D], FP16, kind="ExternalInput"),
        "wb": nc.dram_tensor("wb", [_BF16_TOT], BF16, kind="ExternalInput"),
    }
    out4 = nc.dram_tensor("out4", [GPC, N, D], FP16, kind="ExternalOutput")

    with tile.TileContext(nc) as tc:
        _emit(nc, tc, dram, out4)
    _split_waits(nc)
    return nc


def _wb(dram, name, sz, extra_off=0):
    return dram["wb"][_BF16_OFF[name] + extra_off:
                      _BF16_OFF[name] + extra_off + sz]


def _emit(nc, tc, dram, out4):
    import contextlib
    ctx = contextlib.ExitStack()
    with ctx:
        res = ctx.enter_context(tc.tile_pool(name="res", bufs=1))       # resident
        w1s = ctx.enter_context(tc.tile_pool(name="w1s", bufs=7))       # streamed w1 chunks
        xp = ctx.enter_context(tc.tile_pool(name="xp", bufs=2))
        mkp = ctx.enter_context(tc.tile_pool(name="mkp", bufs=1))
        znat = ctx.enter_context(tc.tile_pool(name="znat", bufs=2))     # z / ao / z2
        ztp = ctx.enter_context(tc.tile_pool(name="ztp", bufs=2))       # zT / aoT / z2T
        qkp = ctx.enter_context(tc.tile_pool(name="qkp", bufs=1))       # qT, kT
        vp = ctx.enter_context(tc.tile_pool(name="vp", bufs=1))
        etp = ctx.enter_context(tc.tile_pool(name="etp", bufs=7))
        gp = ctx.enter_context(tc.tile_pool(name="gp", bufs=1))         # gT resident per graph
        orp = ctx.enter_context(tc.tile_pool(name="orp", bufs=1))       # outres
        fin = ctx.enter_context(tc.tile_pool(name="fin", bufs=2))
        sm = ctx.enter_context(tc.tile_pool(name="sm", bufs=4))         # small stats tiles
        mm = ctx.enter_context(tc.tile_pool(name="mm", bufs=5, space="PSUM"))
        av = ctx.enter_context(tc.tile_pool(name="av", bufs=3, space="PSUM"))

        # ---- input prefetch (graph 0 first so weight loads don't delay it) --
        x_tiles, mk_tiles = {}, {}
        zTs, qTs, kTs, vsbs = {}, {}, {}, {}

        def xsrc(b):
            return dram["x4"][b].rearrange("(t p) d -> p t d", p=128)

        def load_x(b):
            x_tiles[b] = xp.tile([128, NT, D], FP16, name=f"x{b}", tag="x")
            nc.sync.dma_start(out=x_tiles[b], in_=xsrc(b))

        def load_mask(b):
            mk_tiles[b] = mkp.tile([128, NT, N], BF16, name=f"mk{b}", tag="mk")
            nc.sync.dma_start(
                out=mk_tiles[b],
                in_=_wb(dram, "maskT4", N * N, extra_off=b * N * N)
                .rearrange("(c p n) -> p c n", p=128, n=N))

        # ---- one-time loads -------------------------------------------------
        def load_w(name, chunks, cols):
            t = res.tile([128, chunks, cols], BF16, tag=name)
            nc.sync.dma_start(
                out=t, in_=_wb(dram, name, chunks * 128 * cols)
                .rearrange("(c p d) -> p c d", p=128, d=cols))
            return t

        # startup: the serial DMA pipe is the bottleneck, so order it by need:
        # all 4 x tiles, then wq's first column-half (enough for qk j=0..2),
        # then the rest. The zT0 transposes ride the ACT HWDGE queue so their
        # data-waits don't block later SP configs; per-tile LN runs as each x
        # tile lands.
        eps_t = res.tile([128, 1], FP32, tag="eps")
        nc.vector.memset(eps_t, EPS)
        x_tiles[0] = xp.tile([128, NT, D], FP16, name="x0", tag="x")
        x0src = xsrc(0)
        z0_sb = znat.tile([128, NT, D], BF16, tag="znat", name="z0")
        zT0 = ztp.tile([128, DC, N], BF16, tag="zt", name="zT0")
        for t_ in range(NT):
            nc.sync.dma_start(out=x_tiles[0][:, t_, :], in_=x0src[:, t_, :])
        # wq first column-half first (cols j<3 of every dc chunk, enough for
        # qk j=0..2), then the rest after the LN chain is queued
        wq_sb = res.tile([128, DC, D], BF16, tag="wq")
        wq_src = _wb(dram, "wq", DC * 128 * D).rearrange(
            "(c p d) -> p c d", p=128, d=D)
        nc.sync.dma_start(out=wq_sb[:, :, 0:384], in_=wq_src[:, :, 0:384])
        for t_ in range(NT):
            _layernorm_tile(nc, sm, x_tiles[0], z0_sb, eps_t, t_)
            nc.scalar.dma_start_transpose(out=zT0[:, :, t_ * 128:(t_ + 1) * 128],
                                          in_=z0_sb[:, t_, :])
        zTs[0] = zT0
        nc.sync.dma_start(out=wq_sb[:, :, 384:768], in_=wq_src[:, :, 384:768])
        wk_sb = load_w("wk", DC, D)
        wv_sb = load_w("wv", DC, D)
        load_mask(0)

        bq_sb = res.tile([128, DC], BF16, tag="bq")
        nc.sync.dma_start(out=bq_sb,
                          in_=_wb(dram, "bq", D).rearrange("(c p) -> p c", p=128))
        bk_sb = res.tile([128, DC], BF16, tag="bk")
        nc.sync.dma_start(out=bk_sb,
                          in_=_wb(dram, "bk", D).rearrange("(c p) -> p c", p=128))
        b1f_sb = res.tile([128, FC], BF16, tag="b1f")
        nc.sync.dma_start(out=b1f_sb,
                          in_=_wb(dram, "b1f", 4 * D).rearrange("(c p) -> p c", p=128))

        def bcast(name):
            t = res.tile([128, D], BF16, tag=name + "b")
            src = _wb(dram, name, D)
            nc.sync.dma_start(out=t, in_=bass.AP(
                tensor=src.tensor, offset=src.offset, ap=[[0, 128]] + list(src.ap)))
            return t

        bv_b = bcast("bv")
        bo_b = bcast("bo")
        b2_b = bcast("b2f")

        # ---- graph-level software pipeline ---------------------------------
        # LN1/zT and QKV for graph b+1 are emitted inside graph b's body so
        # each engine's static instruction order interleaves next-graph work
        # into this graph's stall windows.

        def ln_z_zT(b):
            z_sb = znat.tile([128, NT, D], BF16, tag="znat", name=f"z{b}")
            zT = ztp.tile([128, DC, N], BF16, tag="zt", name=f"zT{b}")
            _layernorm(nc, sm, x_tiles[b], z_sb, eps_t)
            for t in range(NT):
                nc.sync.dma_start_transpose(
                    out=zT[:, :, t * 128:(t + 1) * 128], in_=z_sb[:, t, :])
            zTs[b] = zT

        def qk_proj(b, split_n=False):
            zT = zTs[b]
            qT = qkp.tile([128, DC, N], BF16, tag="q", name=f"qT{b}")
            kT = qkp.tile([128, DC, N], BF16, tag="k", name=f"kT{b}")
            nsplits = ((0, 256), (256, 512)) if split_n else ((0, 512),)
            jgroups = (range(DC),)
            for jg in jgroups:
                for wsb, bsb, outT in ((wq_sb, bq_sb, qT), (wk_sb, bk_sb, kT)):
                    for j in jg:
                        ps = mm.tile([128, N], FP32, tag="mm")
                        for n0, n1 in nsplits:
                            for dc in range(DC):
                                nc.tensor.matmul(ps[:, n0:n1],
                                                 wsb[:, dc, j * 128:(j + 1) * 128],
                                                 zT[:, dc, n0:n1],
                                                 start=(dc == 0),
                                                 stop=(dc == DC - 1))
                        nc.scalar.activation(out=outT[:, j, :], in_=ps[:, :],
                                             func=F.Identity, bias=bsb[:, j:j + 1])
            qTs[b], kTs[b] = qT, kT

        def v_proj(b):
            zT = zTs[b]
            v_sb = vp.tile([128, NT, H, HD + 1], BF16, name=f"v{b}", tag="v")
            for mc in range(NT):
                for half in range(2):
                    ps = mm.tile([128, 384], FP32, tag="mm")
                    for dc in range(DC):
                        nc.tensor.matmul(ps[:, :],
                                         zT[:, dc, mc * 128:(mc + 1) * 128],
                                         wv_sb[:, dc, half * 384:(half + 1) * 384],
                                         start=(dc == 0), stop=(dc == DC - 1))
                    nc.vector.tensor_tensor(
                        out=v_sb[:, mc, half * 6:(half + 1) * 6, 0:HD],
                        in0=ps.rearrange("p (h d) -> p h d", d=HD),
                        in1=bv_b[:, half * 384:(half + 1) * 384].rearrange(
                            "p (h d) -> p h d", d=HD),
                        op=OP.add)
            nc.vector.memset(v_sb[:, :, :, HD:HD + 1], 1.0)
            vsbs[b] = v_sb

        eTs_pre = {}

        def scores_pair(p, qT, kT, mk_sb, gtag):
            eTp = [etp.tile([128, NT, N], BF16, tag="et", name=f"eT{gtag}_{p}_{s_}")
                   for s_ in range(2)]
            for mc in range(NT):
                for sub in range(2):
                    off = sub * 64
                    ps = mm.tile([128, N], FP32, tag="mm")
                    nc.tensor.matmul(
                        ps[:, :],
                        kT[off:off + 64, p, mc * 128:(mc + 1) * 128],
                        qT[off:off + 64, p, :],
                        start=True, stop=True)
                    nc.scalar.activation(out=eTp[sub][:, mc, :], in_=ps[:, :],
                                         func=F.Exp)
                    nc.vector.tensor_tensor(out=eTp[sub][:, mc, :],
                                            in0=eTp[sub][:, mc, :],
                                            in1=mk_sb[:, mc, :], op=OP.mult)
            return eTp

        qk_proj(0, split_n=True)
        v_proj(0)
        # x1 isn't needed until LN1(1) during graph 0's attention (~30us);
        # keep its transfer out of the startup pipe
        with tc.tile_wait_until(0.012):
            load_x(1)
        # wo/w2 aren't needed until O-proj/FFN2 of graph 0 (~90us in); keep
        # their transfers out of the startup DMA-pipe rush
        with tc.tile_wait_until(0.018):
            wo_sb = load_w("wo", DC, D)
            w2_sb = load_w("w2", FC, D)

        # w1 chunk streaming: deep prefetch on the ACT DMA queue (the SP
        # queue is congested with transposes/stores, which made chunks late)
        W1_LOOKAHEAD = 6

        def load_w1c(fc, tiles, hold=False):
            # SWDGE via gpsimd: stays off the HWDGE generators that serialize
            # the SP/ACT queues' transposes, stores and input loads. `hold`
            # keeps graph 0's prefetches out of the startup DMA-pipe rush
            # (without a dep they'd be scheduled at t=0).
            t = w1s.tile([128, DC, 128], BF16, tag="w1s")
            with tc.tile_wait_until(0.025, enable=hold):
                nc.gpsimd.dma_start(
                    out=t, in_=_wb(dram, "w1", 128 * D, extra_off=fc * 128 * D)
                    .rearrange("(p c f) -> p c f", p=128, f=128))
            tiles.append(t)

        for b in range(GPC):
            x_sb = x_tiles[b]
            mk_sb = mk_tiles.pop(b)
            qT, kT, v_sb = qTs.pop(b), kTs.pop(b), vsbs.pop(b)

            # ---- attention: 6 head-pairs, software-pipelined ----
            ao = znat.tile([128, NT, D], BF16, tag="znat", name=f"ao{b}")
            pav = {}

            def scores_block(p):
                return scores_pair(p, qT, kT, mk_sb, b)

            def av_block(p, eTp):
                half = p // 3
                if half not in pav:
                    pav[half] = [av.tile([128, 6 * (HD + 1)], FP32, tag="av",
                                         name=f"pav{half}_{nc4i}")
                                 for nc4i in range(NT)]
                for sub in range(2):
                    h = 2 * p + sub
                    lane = h - half * 6
                    for nc4 in range(NT):
                        for mc in range(NT):
                            nc.tensor.matmul(
                                pav[half][nc4][:, lane * 65:lane * 65 + 65],
                                eTp[sub][:, mc, nc4 * 128:(nc4 + 1) * 128],
                                v_sb[:, mc, h, :],
                                start=(mc == 0), stop=(mc == NT - 1))

            def evac_block(half):
                for nc4 in range(NT):
                    rec = sm.tile([128, 6], FP32, tag="rec")
                    nc.vector.reciprocal(out=rec, in_=pav[half][nc4][:, HD::HD + 1])
                    rb = bass.AP(tensor=rec.tensor, offset=rec.offset,
                                 ap=[rec.ap[0], [rec.ap[1][0], 6], [0, HD]])
                    nc.vector.tensor_tensor(
                        out=ao[:, nc4, half * 384:(half + 1) * 384].rearrange(
                            "p (h d) -> p h d", d=HD),
                        in0=pav[half][nc4].rearrange(
                            "p (h s) -> p h s", s=HD + 1)[:, :, 0:HD],
                        in1=rb, op=OP.mult)

            pre = eTs_pre.pop(b, None)
            if pre:
                queue = list(pre)
                nxt = len(pre)
            else:
                queue = [scores_block(0)]
                nxt = 1
            for p in range(6):
                if nxt < 6:
                    queue.append(scores_block(nxt))
                    nxt += 1
                eT_cur = queue.pop(0)
                av_block(p, eT_cur)
                if p == 0 and b + 1 < GPC:
                    # next graph's LN1+transpose: its DVE/ACT/DMA work fills
                    # this graph's attention slack, and zT(b+1) is ready before
                    # the post-attention QK(b+1) matmuls need it.
                    ln_z_zT(b + 1)
                if p == 2:
                    evac_block(0)
                elif p == 5:
                    evac_block(1)
                    if b + 1 < GPC:
                        load_mask(b + 1)

            # x += bo (residual bias; gpsimd is free again here)
            for t in range(NT):
                nc.gpsimd.tensor_tensor(out=x_sb[:, t, :], in0=x_sb[:, t, :],
                                        in1=bo_b, op=OP.add)

            # aoT
            aoT = ztp.tile([128, DC, N], BF16, tag="zt", name=f"aoT{b}")
            for t in range(NT):
                nc.sync.dma_start_transpose(out=aoT[:, :, t * 128:(t + 1) * 128],
                                            in_=ao[:, t, :])

            # next graph's q/k projections fill the aoT-transpose latency
            if b + 1 < GPC:
                qk_proj(b + 1)

            # prefetch first w1 chunks (Pool SWDGE queue)
            w1_tiles = []
            for fc in range(W1_LOOKAHEAD):
                load_w1c(fc, w1_tiles, hold=(b == 0))

            # O projection + residual -> outres (fp16), LN2 interleaved per
            # tile so z2T(t) transposes overlap O-proj of tiles t+1..3 (the
            # DVE/ACT LN2 work for tile t queues before tile t+1's evac).
            outres = orp.tile([128, NT, D], FP16, name=f"or{b}", tag="or")
            sums = sm.tile([128, NT, 2], FP32, tag="s1")
            sumsq = sm.tile([128, NT, 2], FP32, tag="s2")
            z2 = znat.tile([128, NT, D], BF16, tag="znat", name=f"z2{b}")
            z2T = ztp.tile([128, DC, N], BF16, tag="zt", name=f"z2T{b}")
            for nc4 in range(NT):
                for half in range(2):
                    ps = mm.tile([128, 384], FP32, tag="mm")
                    for dc in range(DC):
                        nc.tensor.matmul(ps[:, :],
                                         aoT[:, dc, nc4 * 128:(nc4 + 1) * 128],
                                         wo_sb[:, dc, half * 384:(half + 1) * 384],
                                         start=(dc == 0), stop=(dc == DC - 1))
                    # evacuation + residual, with a free row-sum for LN2 stats
                    nc.vector.scalar_tensor_tensor(
                        out=outres[:, nc4, half * 384:(half + 1) * 384],
                        in0=ps[:, :], scalar=0.0,
                        in1=x_sb[:, nc4, half * 384:(half + 1) * 384],
                        op0=OP.add, op1=OP.add,
                        accum_out=sums[:, nc4, half:half + 1])
                    # row sum of squares on the scalar engine (idle here)
                    sq = fin.tile([128, 384], FP32, tag="fin")
                    nc.scalar.activation(
                        out=sq, in_=outres[:, nc4, half * 384:(half + 1) * 384],
                        func=F.Square,
                        accum_out=sumsq[:, nc4, half:half + 1])
                # LN2(tile nc4): mu = s/D, var = sq/D - mu^2
                t = nc4
                mu = sm.tile([128, 1], FP32, tag="mv")
                nc.vector.tensor_reduce(out=mu, in_=sums[:, t, :],
                                        axis=mybir.AxisListType.X, op=OP.add)
                nc.vector.tensor_scalar(out=mu, in0=mu, scalar1=1.0 / D,
                                        scalar2=None, op0=OP.mult)
                ex2 = sm.tile([128, 1], FP32, tag="lnv")
                nc.vector.tensor_reduce(out=ex2, in_=sumsq[:, t, :],
                                        axis=mybir.AxisListType.X, op=OP.add)
                nc.vector.tensor_scalar(out=ex2, in0=ex2, scalar1=1.0 / D,
                                        scalar2=None, op0=OP.mult)
                negvar = sm.tile([128, 1], FP32, tag="bn")
                nc.vector.scalar_tensor_tensor(
                    out=negvar, in0=mu, scalar=mu, in1=ex2,
                    op0=OP.mult, op1=OP.subtract)   # mu^2 - E[x^2] = -var
                lnv = sm.tile([128, 1], FP32, tag="lnv2")
                # Ln(-1 * negvar + eps) = Ln(var + eps)
                nc.scalar.activation(out=lnv, in_=negvar, func=F.Ln, bias=eps_t,
                                     scale=-1.0)
                rstd = sm.tile([128, 1], FP32, tag="rstd")
                nc.scalar.activation(out=rstd, in_=lnv, func=F.Exp, scale=-0.5)
                nc.vector.tensor_scalar(out=z2[:, t, :], in0=outres[:, t, :],
                                        scalar1=mu, scalar2=rstd,
                                        op0=OP.subtract, op1=OP.mult)
                # ACT queue: fires right after this tile's rstd instead of
                # queuing behind unrelated SP configs
                nc.scalar.dma_start_transpose(
                    out=z2T[:, :, t * 128:(t + 1) * 128], in_=z2[:, t, :])

            # next graph's v projection fills the LN2/z2T stall window on the PE
            if b + 1 < GPC:
                v_proj(b + 1)

            # outres += b2 (final-residual bias), off critical path
            for t in range(NT):
                nc.gpsimd.tensor_tensor(out=outres[:, t, :], in0=outres[:, t, :],
                                        in1=b2_b, op=OP.add)

            # FFN1 + gelu -> gT. First chunks run as two n-halves so they only
            # need z2T transposes t=0,1 (t=2,3 still in flight after O-proj).
            gT = gp.tile([128, FC, N], BF16, name=f"gT{b}", tag="gT")
            for fc in range(FC):
                if fc + W1_LOOKAHEAD < FC:
                    load_w1c(fc + W1_LOOKAHEAD, w1_tiles,
                             hold=(b == 0 and fc < 2))
                w1c = w1_tiles[fc]
                ps = mm.tile([128, N], FP32, tag="mm")
                nsp = ((0, 256), (256, 512)) if fc < 4 else ((0, 512),)
                for n0, n1 in nsp:
                    for dc in range(DC):
                        nc.tensor.matmul(ps[:, n0:n1], w1c[:, dc, :],
                                         z2T[:, dc, n0:n1],
                                         start=(dc == 0), stop=(dc == DC - 1))
                nc.scalar.activation(out=gT[:, fc, :], in_=ps[:, :], func=F.Gelu,
                                     bias=b1f_sb[:, fc:fc + 1])
                if b + 1 < GPC and fc in (15, 19):
                    # pre-compute next graph's first score pairs in the FFN
                    # window (ACT has slack beside gelu); the next attention
                    # window then starts PE-bound instead of exp-bound
                    eTs_pre.setdefault(b + 1, []).append(
                        scores_pair(0 if fc == 15 else 1, qTs[b + 1],
                                    kTs[b + 1], mk_tiles[b + 1], b + 1))

            if b + 2 < GPC:
                load_x(b + 2)

            # FFN2 + final residual -> out. ACT is idle here, so pre-compute
            # two more of the next graph's score pairs (exp on ACT) — the
            # next attention window then starts with 4/6 pairs done.
            for nc4 in range(NT):
                for half in range(2):
                    ps = mm.tile([128, 384], FP32, tag="mm")
                    for fc in range(FC):
                        nc.tensor.matmul(ps[:, :],
                                         gT[:, fc, nc4 * 128:(nc4 + 1) * 128],
                                         w2_sb[:, fc, half * 384:(half + 1) * 384],
                                         start=(fc == 0), stop=(fc == FC - 1))
                    ft = fin.tile([128, 384], FP16, tag="fin")
                    nc.vector.tensor_tensor(
                        out=ft, in0=ps[:, :],
                        in1=outres[:, nc4, half * 384:(half + 1) * 384], op=OP.add)
                    nc.sync.dma_start(
                        out=out4[b, nc4 * 128:(nc4 + 1) * 128,
                                 half * 384:(half + 1) * 384],
                        in_=ft)
                if b + 1 < GPC and nc4 in (1, 3):
                    eTs_pre[b + 1].append(
                        scores_pair(2 if nc4 == 1 else 3, qTs[b + 1],
                                    kTs[b + 1], mk_tiles[b + 1], b + 1))
            del x_tiles[b]


def _layernorm_tile(nc, sm, src, dst, eps_t, t):
    """Single-tile LN chain (stats -> rstd -> z) for pipeline fill: no
    cross-tile batching, so tile t's output is ready as soon as tile t's
    input is."""
    stats = sm.tile([128, 3, 6], FP32, tag="bn")
    xg = src[:, t, :].rearrange("p (s d) -> p s d", s=3)
    for s in range(3):
        nc.vector.bn_stats(out=stats[:, s, :], in_=xg[:, s, :])
    mv = sm.tile([128, 2], FP32, tag="mv")
    nc.vector.bn_aggr(out=mv, in_=stats)
    lnv = sm.tile([128, 1], FP32, tag="lnv")
    nc.scalar.activation(out=lnv, in_=mv[:, 1:2], func=F.Ln, bias=eps_t)
    rstd = sm.tile([128, 1], FP32, tag="rstd")
    nc.scalar.activation(out=rstd, in_=lnv, func=F.Exp, scale=-0.5)
    nc.vector.tensor_scalar(out=dst[:, t, :], in0=src[:, t, :],
                            scalar1=mv[:, 0:1], scalar2=rstd,
                            op0=OP.subtract, op1=OP.mult)


def _layernorm(nc, sm, src, dst, eps_t):
    """src [128, NT, 768] fp32 -> dst [128, NT, 768] bf16, per-row LN without
    affine (folded into downstream weights). The Ln/Exp rstd ops are batched
    across all NT tiles (2 ACT ops instead of 2*NT) since this runs inside the
    ACT-bound attention window."""
    mv4 = sm.tile([128, NT, 2], FP32, tag="mv")
    for t in range(NT):
        stats = sm.tile([128, 3, 6], FP32, tag="bn")
        xg = src[:, t, :].rearrange("p (s d) -> p s d", s=3)
        for s in range(3):
            nc.vector.bn_stats(out=stats[:, s, :], in_=xg[:, s, :])
        nc.vector.bn_aggr(out=mv4[:, t, :], in_=stats)
    lnv4 = sm.tile([128, NT], FP32, tag="lnv")
    nc.scalar.activation(out=lnv4, in_=mv4[:, :, 1], func=F.Ln, bias=eps_t)
    rstd4 = sm.tile([128, NT], FP32, tag="rstd")
    nc.scalar.activation(out=rstd4, in_=lnv4, func=F.Exp, scale=-0.5)
    for t in range(NT):
        nc.vector.tensor_scalar(out=dst[:, t, :], in0=src[:, t, :],
                                scalar1=mv4[:, t, 0:1], scalar2=rstd4[:, t:t + 1],
                                op0=OP.subtract, op1=OP.mult)


_CACHE = {}


def _get_module():
    if "nc" not in _CACHE:
        _CACHE["nc"] = _build_module()
    return _CACHE["nc"]


def _prep_inputs(inputs):
    x = np.ascontiguousarray(np.asarray(inputs["x"], dtype=np.float32)
                             .astype(np.float16))
    adj = np.asarray(inputs["adj"])
    g1 = np.asarray(inputs["g1"], dtype=np.float32)
    b1 = np.asarray(inputs["b1"], dtype=np.float32)
    g2 = np.asarray(inputs["g2"], dtype=np.float32)
    b2 = np.asarray(inputs["b2"], dtype=np.float32)
    scale = HD ** -0.5

    wq = np.asarray(inputs["wq"], np.float32)
    wk = np.asarray(inputs["wk"], np.float32)
    wv = np.asarray(inputs["wv"], np.float32)
    w1 = np.asarray(inputs["w_ffn1"], np.float32)

    consts = {
        "wq": ((g1[:, None] * wq) * scale).astype(bf16),
        "wk": (g1[:, None] * wk).astype(bf16),
        "wv": (g1[:, None] * wv).astype(bf16),
        "wo": np.asarray(inputs["wo"], np.float32).astype(bf16),
        "w1": (g2[:, None] * w1).astype(bf16).reshape(6, 128, 24, 128)
              .transpose(2, 1, 0, 3).reshape(24, 128, 768),
        "w2": np.asarray(inputs["w_ffn2"], np.float32).astype(bf16),
        "bq": ((b1 @ wq + np.asarray(inputs["bq"], np.float32)) * scale),
        "bk": (b1 @ wk + np.asarray(inputs["bk"], np.float32)),
        "bv": (b1 @ wv + np.asarray(inputs["bv"], np.float32)).astype(bf16),
        "bo": np.asarray(inputs["bo"], np.float32).astype(bf16),
        "b1f": (b2 @ w1 + np.asarray(inputs["b_ffn1"], np.float32)),
        "b2f": np.asarray(inputs["b_ffn2"], np.float32).astype(bf16),
    }
    maskT = np.ascontiguousarray(adj.transpose(0, 2, 1)).astype(bf16)
    for nm in ("bq", "bk", "b1f"):
        consts[nm] = consts[nm].astype(bf16)

    # assemble the single blob (see _BF16_SPEC)
    wb_const = np.concatenate(
        [np.ascontiguousarray(consts[nm]).ravel()
         for nm, _ in _BF16_SPEC if nm not in ("maskT4", "x4")])

    in_maps = []
    for c in range(NCORES):
        wb = np.concatenate(
            [wb_const, maskT[c * GPC:(c + 1) * GPC].ravel()])
        assert wb.size == _BF16_TOT
        in_maps.append({"x4": x[c * GPC:(c + 1) * GPC], "wb": wb})
    return in_maps


def _run(inputs, **kwargs):
    nc = _get_module()
    in_maps = _prep_inputs(inputs)
    res = run_bass_kernel_spmd(nc, in_maps, core_ids=list(range(NCORES)), **kwargs)
    out = np.concatenate([res.results[c]["out4"] for c in range(NCORES)],
                         axis=0).astype(np.float32)
    return out, res


def kernel(**inputs) -> np.ndarray:
    out, _ = _run(inputs)
    return out


def run_traced(inputs):
    """For test.py: returns (output, BassKernelResults with profile info)."""
    return _run(inputs, trace=True)

